# revision 5
# baseline (speedup 1.0000x reference)
"""Trainium2 Bass kernel for nn_LogMCDropoutVariance.

Computes, for inputs features[B,F], W[F,C], b[C] (logits input unused):
    masks  = bernoulli(key42, 0.9, (S,B,F))          (constant, host-precomputed)
    lgts_s = (features * mask_s / 0.9) @ W + b
    p_s    = softmax(lgts_s)
    out    = log( sum_c var_s(p) + 1e-12 )           -> [B, 1]

Strategy: pure data-parallel over batch across 8 NeuronCores. On device:
bf16 matmuls (mask applied as bf16 0/1 multiply on transposed features),
softmax via ACT Exp (fp16) + fused DVE reduce for the exp(b)-weighted
denominator, variance via the  sum_s sum_c p^2 - S * sum_c mean^2  identity
(per-sample sum p^2 from ACT Square accum_out).  Device returns
tv_pre = B1 - B2/10 per row; host applies log(tv_pre/9 + eps).
"""

import sys

if "/opt/trn_rl_repo" not in sys.path:
    sys.path.insert(0, "/opt/trn_rl_repo")

import numpy as np
import ml_dtypes

S = 10
B, F, C = 32768, 512, 1000
KEEP = 0.9
EPS = 1e-12
P = 128
KCH = F // P            # 4 contraction chunks
NCORES = 8
BSH = B // NCORES       # 4096 rows per core
RT = BSH // P           # 32 row tiles per core
CSPLIT = ((0, 512), (512, 488))  # matmul N-chunks, each within one PSUM bank

_cache: dict = {}


def _host_masks():
    """Bit-exact reproduction of the reference's dropout masks, transposed to
    [s, rt_global, p, k, r] where mask[s, 128*rt + r, 128*k + p] lives at
    [s, rt, p, k, r]; bf16 0/1 so the device multiply is exact."""
    import jax

    with jax.default_device(jax.devices("cpu")[0]):
        masks = np.asarray(
            jax.random.bernoulli(jax.random.key(42), KEEP, (S, B, F))
        )
    mt = masks.reshape(S, B // P, P, KCH, P).transpose(0, 1, 4, 3, 2)
    return np.ascontiguousarray(mt).astype(ml_dtypes.bfloat16)


def _emit(tc, x_d, mt_d, w_d, eb_d, tv_d, n_rt):
    import concourse.bass as bass
    import concourse.tile as tile  # noqa: F401
    from concourse import mybir
    from concourse.masks import make_identity
    from concourse.dve_ops import TENSOR_TENSOR_REDUCE
    from contextlib import ExitStack

    # out = in0*in1 ; accum_out = sum(out). Uses the table-shipped custom-DVE
    # op (the raw TENSOR_TENSOR_REDUCE isa opcode crashes this terminal's fw).
    def ttr(nc, out, in0, in1, accum_out):
        nc.vector._custom_dve(
            TENSOR_TENSOR_REDUCE,
            out=out,
            in0=in0,
            in1=in1,
            s0=0.0,
            s1=1.0,
            accum_out=accum_out,
        )

    nc = tc.nc
    dt = mybir.dt
    Alu = mybir.AluOpType
    Act = mybir.ActivationFunctionType

    with ExitStack() as ctx:
        singles = ctx.enter_context(tc.tile_pool(name="singles", bufs=1))
        xpool = ctx.enter_context(tc.tile_pool(name="xpool", bufs=3))
        xtpool = ctx.enter_context(tc.tile_pool(name="xtpool", bufs=3))
        mpool = ctx.enter_context(tc.tile_pool(name="mpool", bufs=8))
        dpool = ctx.enter_context(tc.tile_pool(name="dpool", bufs=4))
        upool = ctx.enter_context(tc.tile_pool(name="upool", bufs=4))
        ubpool = ctx.enter_context(tc.tile_pool(name="ubpool", bufs=4))
        dumppool = ctx.enter_context(tc.tile_pool(name="dumppool", bufs=3))
        maccpool = ctx.enter_context(tc.tile_pool(name="maccpool", bufs=2))
        statpool = ctx.enter_context(tc.tile_pool(name="statpool", bufs=4))
        psum_lgt = ctx.enter_context(
            tc.tile_pool(name="psum_lgt", bufs=3, space="PSUM")
        )
        psum_xt = ctx.enter_context(
            tc.tile_pool(name="psum_xt", bufs=2, space="PSUM")
        )

        # constants
        ident = singles.tile([P, P], dt.bfloat16)
        make_identity(nc, ident)
        w_sb = singles.tile([P, KCH, C], dt.bfloat16)
        nc.sync.dma_start(w_sb, w_d.rearrange("k p c -> p k c"))
        eb_sb = singles.tile([P, C], dt.float16)
        nc.sync.dma_start(
            eb_sb,
            bass.AP(tensor=eb_d.tensor, offset=eb_d.offset, ap=[[0, P], [1, C]]),
        )

        for rt in range(n_rt):
            # load features tile, cast to bf16, transpose via PE
            x_tile = xpool.tile([P, F], dt.float32)
            nc.sync.dma_start(x_tile, x_d[rt * P : (rt + 1) * P, :])
            xb_tile = xpool.tile([P, F], dt.bfloat16)
            nc.vector.tensor_copy(xb_tile, x_tile)
            xt_sb = xtpool.tile([P, KCH, P], dt.bfloat16)
            for k in range(KCH):
                pt = psum_xt.tile([P, P], dt.bfloat16)
                nc.tensor.transpose(pt, xb_tile[:, k * P : (k + 1) * P], ident)
                nc.vector.tensor_copy(xt_sb[:, k, :], pt)

            m_acc = maccpool.tile([P, C], dt.float16)
            zcol = statpool.tile([P, S], dt.float32)
            qcol = statpool.tile([P, S], dt.float32)
            rzcol = statpool.tile([P, S], dt.float32)

            for s in range(S):
                mk = mpool.tile([P, KCH, P], dt.bfloat16)
                nc.sync.dma_start(mk, mt_d[s, rt])
                dr = dpool.tile([P, KCH, P], dt.bfloat16)
                nc.vector.tensor_mul(dr, xt_sb, mk)

                lg = psum_lgt.tile([P, C], dt.float32)
                for k in range(KCH):
                    for c0, cw in CSPLIT:
                        nc.tensor.matmul(
                            lg[:, c0 : c0 + cw],
                            lhsT=dr[:, k, :],
                            rhs=w_sb[:, k, c0 : c0 + cw],
                            start=(k == 0),
                            stop=(k == KCH - 1),
                        )

                u = upool.tile([P, C], dt.float16)
                nc.scalar.activation(u, lg, Act.Exp)
                # ub = u * exp(b);  Z_s = sum_c ub
                ub = ubpool.tile([P, C], dt.float16)
                ttr(nc, ub, u, eb_sb, zcol[:, s : s + 1])
                nc.vector.reciprocal(rzcol[:, s : s + 1], zcol[:, s : s + 1])
                # m_acc += ub * (1/Z_s)   (= sum_s p_s)
                if s == 0:
                    nc.vector.tensor_scalar(
                        m_acc, ub, rzcol[:, 0:1], None, op0=Alu.mult
                    )
                else:
                    nc.vector.scalar_tensor_tensor(
                        out=m_acc,
                        in0=ub,
                        scalar=rzcol[:, s : s + 1],
                        in1=m_acc,
                        op0=Alu.mult,
                        op1=Alu.add,
                    )
                # qtil_s = sum_c ub^2   (-> sum_c p^2 = qtil_s / Z_s^2)
                dump = dumppool.tile([P, C], dt.float32)
                nc.scalar.activation(
                    dump, ub, Act.Square, accum_out=qcol[:, s : s + 1]
                )

            # tail: B1 = sum_s qtil_s / Z_s^2 ; B2 = sum_c m_acc^2
            rsq = statpool.tile([P, S], dt.float32)
            nc.vector.tensor_mul(rsq, rzcol, rzcol)
            dump10 = statpool.tile([P, S], dt.float32)
            b1 = statpool.tile([P, 1], dt.float32)
            ttr(nc, dump10, qcol, rsq, b1)
            dumpc = dumppool.tile([P, C], dt.float16)
            b2 = statpool.tile([P, 1], dt.float32)
            ttr(nc, dumpc, m_acc, m_acc, b2)
            # tv_pre = B1 - B2/10
            tv = statpool.tile([P, 1], dt.float32)
            nc.vector.scalar_tensor_tensor(
                out=tv,
                in0=b2,
                scalar=-0.1,
                in1=b1,
                op0=Alu.mult,
                op1=Alu.add,
            )
            nc.sync.dma_start(tv_d[rt * P : (rt + 1) * P, :], tv)


def _build(n_rt=RT):
    from concourse import bacc, mybir
    import concourse.tile as tile

    dt = mybir.dt
    nc = bacc.Bacc(
        "TRN2", target_bir_lowering=False, debug=False, enable_asserts=False
    )
    x_d = nc.dram_tensor("x", [BSH, F], dt.float32, kind="ExternalInput").ap()
    mt_d = nc.dram_tensor(
        "mt", [S, RT, P, KCH, P], dt.bfloat16, kind="ExternalInput"
    ).ap()
    w_d = nc.dram_tensor("w", [KCH, P, C], dt.bfloat16, kind="ExternalInput").ap()
    eb_d = nc.dram_tensor("eb", [C], dt.float16, kind="ExternalInput").ap()
    tv_d = nc.dram_tensor("tv", [BSH, 1], dt.float32, kind="ExternalOutput").ap()
    with tile.TileContext(nc) as tc:
        _emit(tc, x_d, mt_d, w_d, eb_d, tv_d, n_rt)
    nc.compile()
    return nc


def _prep_host(features, W, b):
    if "mt" not in _cache:
        _cache["mt"] = _host_masks()
    mt = _cache["mt"]
    x32 = np.ascontiguousarray(np.asarray(features, dtype=np.float32))
    w_bf = (np.asarray(W, dtype=np.float32) / np.float32(KEEP)).astype(
        ml_dtypes.bfloat16
    )
    w_bf = np.ascontiguousarray(w_bf.reshape(KCH, P, C))
    eb = np.exp(np.asarray(b, dtype=np.float64)).astype(np.float16)
    in_maps = []
    for i in range(NCORES):
        in_maps.append(
            {
                "x": x32[i * BSH : (i + 1) * BSH],
                "mt": np.ascontiguousarray(
                    mt[:, i * RT : (i + 1) * RT]
                ),
                "w": w_bf,
                "eb": eb,
            }
        )
    return in_maps


def _make_runner(nc):
    """Build a cached jitted shard_map callable running the NEFF on 8 cores.

    Mirrors bass2jax.run_bass_via_pjrt's multi-core path, but reusable across
    calls (run_bass_via_pjrt re-traces and re-jits on every invocation)."""
    import jax
    from jax.experimental.shard_map import shard_map
    from jax.sharding import Mesh, PartitionSpec
    from concourse import bass2jax, mybir

    bass2jax.install_neuronx_cc_hook()

    partition_name = (
        nc.partition_id_tensor.name if nc.partition_id_tensor else None
    )
    in_names, out_names, out_avals, zero_shapes = [], [], [], []
    for alloc in nc.m.functions[0].allocations:
        if not isinstance(alloc, mybir.MemoryLocationSet):
            continue
        name = alloc.memorylocations[0].name
        if alloc.kind == "ExternalInput":
            if name != partition_name:
                in_names.append(name)
        elif alloc.kind == "ExternalOutput":
            assert alloc.tensor_shape is not None and alloc.dtype is not None
            out_names.append(name)
            shape = tuple(alloc.tensor_shape)
            np_dt = mybir.dt.np(alloc.dtype)
            out_avals.append(jax.core.ShapedArray(shape, np_dt))
            zero_shapes.append((shape, np_dt))
    n_params = len(in_names)
    n_outs = len(out_names)
    all_in_names = list(in_names) + list(out_names)
    if partition_name is not None:
        all_in_names.append(partition_name)

    def _body(*args):
        operands = list(args)
        if partition_name is not None:
            operands.append(bass2jax.partition_id_tensor())
        outs = bass2jax._bass_exec_p.bind(
            *operands,
            out_avals=tuple(out_avals),
            in_names=tuple(all_in_names),
            out_names=tuple(out_names),
            lowering_input_output_aliases=(),
            sim_require_finite=True,
            sim_require_nnan=True,
            nc=nc,
        )
        return tuple(outs)

    devices = jax.devices()[:NCORES]
    mesh = Mesh(np.asarray(devices), ("core",))
    sharded = jax.jit(
        shard_map(
            _body,
            mesh=mesh,
            in_specs=(PartitionSpec("core"),) * (n_params + n_outs),
            out_specs=(PartitionSpec("core"),) * n_outs,
            check_rep=False,
        ),
        keep_unused=True,
    )
    return {
        "fn": sharded,
        "in_names": in_names,
        "out_names": out_names,
        "zero_shapes": zero_shapes,
        "mesh": mesh,
    }


def _runner():
    if "runner" not in _cache:
        if "nc" not in _cache:
            _cache["nc"] = _build()
        _cache["runner"] = _make_runner(_cache["nc"])
    return _cache["runner"]


def _concat_inputs(in_maps, runner):
    concat = [
        np.concatenate([m[name] for m in in_maps], axis=0)
        for name in runner["in_names"]
    ]
    zeros = [
        np.zeros((NCORES * sh[0],) + tuple(sh[1:]), dt)
        for sh, dt in runner["zero_shapes"]
    ]
    return concat, zeros


def kernel(features, logits, W, b):
    del logits
    r = _runner()
    in_maps = _prep_host(features, W, b)
    concat, zeros = _concat_inputs(in_maps, r)
    outs = r["fn"](*concat, *zeros)
    tv_pre = np.asarray(outs[r["out_names"].index("tv")])
    total_var = np.maximum(tv_pre, 0.0).astype(np.float64) / (S - 1)
    return np.log(total_var + EPS).astype(np.float32)
